# revision 1
# baseline (speedup 1.0000x reference)
"""Diagonal SSM kernel for 8 Trainium2 NeuronCores.

Math (per batch element b, sharded one per core):
    alpha = sigmoid(u @ Wa.T + ba)          (S, N)
    Bu    = u @ Wb.T + bb                   (S, N)
    x_t   = alpha_t * x_{t-1} + Bu_t        (scan over S)
    y     = xs @ C.T + u @ Dm.T             (S, D)

Device strategy (per core):
  - u (S, D) fp32 is cast to bf16 via SWDGE cast-DMA into a DRAM scratch,
    then DMA-transposed (hardware xbar, 2-byte) into SBUF as uT [D x S] bf16.
  - GEMM-A: psum[n, s-chunk] = sum_d WabT[d, n-tile] . uT[d, s-chunk]
    (Wa and Wb stacked into one 512-row weight so alpha/Bu share the loop);
    ScalarE applies sigmoid(+ba) / identity(+bb) straight out of PSUM.
  - Recurrence: native VectorE tensor_tensor_scan (op0=mult, op1=add,
    fp32 internal state) along the free dim, chunk-chained via a
    per-partition initial value.
  - GEMM-B: y[s-tile, d] = xsT.T @ CT + uT.T @ DmT accumulated in PSUM,
    copied to SBUF (DVE/ACT alternating) and DMA'd out as fp32.

Params are pre-packed on host (transposed, bf16) - standard weight packing.
The full u tensor is read on device in fp32.
"""

import numpy as np
import ml_dtypes

B, S, D, N = 8, 4096, 1024, 256
NCORES = 8
KT = D // 128          # 8 contraction tiles
SC = 512               # s-chunk (matmul free dim / PSUM bank)
NSC = S // SC          # 8 s-chunks
TC = 1024              # transpose/cast chunk (rows of u)

_CACHE = {}
LAST_RESULTS = None    # test harness reads profiling info from here


def _build_program():
    import concourse.mybir as mybir
    import concourse.tile as tile
    from concourse import bacc

    fp32 = mybir.dt.float32
    bf16 = mybir.dt.bfloat16
    AF = mybir.ActivationFunctionType
    OP = mybir.AluOpType

    nc = bacc.Bacc(
        "TRN2",
        target_bir_lowering=False,
        debug=False,
        enable_asserts=False,
        num_devices=NCORES,
    )

    u = nc.dram_tensor("u", [S, D], fp32, kind="ExternalInput").ap()
    wabt = nc.dram_tensor("wabt", [D, 2 * N], bf16, kind="ExternalInput").ap()
    bias = nc.dram_tensor("bias", [128, 4], fp32, kind="ExternalInput").ap()
    ct = nc.dram_tensor("ct", [N, D], bf16, kind="ExternalInput").ap()
    dmt = nc.dram_tensor("dmt", [D, D], bf16, kind="ExternalInput").ap()
    y = nc.dram_tensor("y", [S, D], fp32, kind="ExternalOutput").ap()

    with tile.TileContext(nc) as tc:
        with (
            tc.tile_pool(name="consts", bufs=1) as consts,
            tc.tile_pool(name="dram", bufs=1, space="DRAM") as dpool,
            tc.tile_pool(name="data", bufs=1) as data,
            tc.tile_pool(name="psA", bufs=3, space="PSUM") as psA,
            tc.tile_pool(name="psB", bufs=4, space="PSUM") as psB,
            tc.tile_pool(name="ypool", bufs=3) as ypool,
        ):
            # ---- params into SBUF ----
            wabt_sb = [consts.tile([128, 2 * N], bf16, name=f"wabt{k}") for k in range(KT)]
            for k in range(KT):
                nc.sync.dma_start(out=wabt_sb[k][:], in_=wabt[k * 128:(k + 1) * 128, :])
            ct_sb = [consts.tile([128, D], bf16, name=f"ct{h}") for h in range(2)]
            for h in range(2):
                nc.sync.dma_start(out=ct_sb[h][:], in_=ct[h * 128:(h + 1) * 128, :])
            dmt_sb = [consts.tile([128, D], bf16, name=f"dmt{k}") for k in range(KT)]
            for k in range(KT):
                nc.sync.dma_start(out=dmt_sb[k][:], in_=dmt[k * 128:(k + 1) * 128, :])
            bias_sb = consts.tile([128, 4], fp32, name="bias_sb")
            nc.sync.dma_start(out=bias_sb[:], in_=bias[:])

            # ---- u: cast to bf16 (DRAM scratch) then xbar-transpose into SBUF ----
            u16 = dpool.tile([S, D], bf16, name="u16")
            uT = [data.tile([128, S], bf16, name=f"uT{k}") for k in range(KT)]
            for c in range(S // TC):
                rs = slice(c * TC, (c + 1) * TC)
                nc.gpsimd.dma_start(out=u16[rs, :], in_=u[rs, :])  # SWDGE cast
                for k in range(KT):
                    nc.sync.dma_start(
                        out=uT[k][:, rs],
                        in_=u16[rs, k * 128:(k + 1) * 128],
                        transpose=True,
                    )

            alphaT = [data.tile([128, S], bf16, name=f"alphaT{h}") for h in range(2)]
            buT = [data.tile([128, S], bf16, name=f"buT{h}") for h in range(2)]
            xsT = [data.tile([128, S], bf16, name=f"xsT{h}") for h in range(2)]

            for sc in range(NSC):
                ssl = slice(sc * SC, (sc + 1) * SC)
                # ---- GEMM-A: alphaT / BuT for this s-chunk ----
                for nt in range(4):
                    ps = psA.tile([128, SC], fp32, name="psa", tag="psa")
                    for k in range(KT):
                        nc.tensor.matmul(
                            ps[:],
                            wabt_sb[k][:, nt * 128:(nt + 1) * 128],
                            uT[k][:, ssl],
                            start=(k == 0),
                            stop=(k == KT - 1),
                        )
                    if nt < 2:
                        nc.scalar.activation(
                            alphaT[nt][:, ssl], ps[:], AF.Sigmoid,
                            bias=bias_sb[:, nt:nt + 1],
                        )
                    else:
                        nc.scalar.activation(
                            buT[nt - 2][:, ssl], ps[:], AF.Identity,
                            bias=bias_sb[:, nt:nt + 1],
                        )
                # ---- scan (chained across chunks via last column) ----
                for h in range(2):
                    init = 0.0 if sc == 0 else xsT[h][:, sc * SC - 1:sc * SC]
                    nc.vector.tensor_tensor_scan(
                        xsT[h][:, ssl],
                        alphaT[h][:, ssl],
                        buT[h][:, ssl],
                        init,
                        op0=OP.mult,
                        op1=OP.add,
                    )
                # ---- GEMM-B: y for the 4 s-tiles of this chunk ----
                for t in range(4):
                    st = sc * 4 + t
                    stsl = slice(st * 128, (st + 1) * 128)
                    ytile = ypool.tile([128, D], fp32, name="ytile", tag="ytile")
                    for dc in range(2):
                        dsl = slice(dc * SC, (dc + 1) * SC)
                        ps = psB.tile([128, SC], fp32, name="psb", tag="psb")
                        nc.tensor.matmul(ps[:], xsT[0][:, stsl], ct_sb[0][:, dsl],
                                         start=True, stop=False)
                        nc.tensor.matmul(ps[:], xsT[1][:, stsl], ct_sb[1][:, dsl],
                                         start=False, stop=False)
                        for k in range(KT):
                            nc.tensor.matmul(ps[:], uT[k][:, stsl], dmt_sb[k][:, dsl],
                                             start=False, stop=(k == KT - 1))
                        if dc == 0:
                            nc.vector.tensor_copy(ytile[:, dsl], ps[:])
                        else:
                            nc.scalar.copy(ytile[:, dsl], ps[:])
                    nc.sync.dma_start(out=y[stsl, :], in_=ytile[:])

    nc.compile()
    return nc


def _get_program():
    if "nc" not in _CACHE:
        _CACHE["nc"] = _build_program()
    return _CACHE["nc"]


def kernel(u, Wa, ba, Wb, bb, C, Dm):
    global LAST_RESULTS
    from concourse.bass_utils import run_bass_kernel_spmd

    nc = _get_program()

    u = np.asarray(u, dtype=np.float32)
    bf = ml_dtypes.bfloat16
    wabt_np = np.ascontiguousarray(
        np.concatenate([np.asarray(Wa), np.asarray(Wb)], axis=0).T
    ).astype(bf)                                                   # (D, 2N)
    bias_np = np.ascontiguousarray(
        np.concatenate([np.asarray(ba), np.asarray(bb)]).astype(np.float32)
        .reshape(4, 128).T
    )                                                              # (128, 4)
    ct_np = np.ascontiguousarray(np.asarray(C).T).astype(bf)       # (N, D)
    dmt_np = np.ascontiguousarray(np.asarray(Dm).T).astype(bf)     # (D, D)

    in_maps = [
        {
            "u": np.ascontiguousarray(u[b]),
            "wabt": wabt_np,
            "bias": bias_np,
            "ct": ct_np,
            "dmt": dmt_np,
        }
        for b in range(B)
    ]

    res = run_bass_kernel_spmd(nc, in_maps, core_ids=list(range(NCORES)))
    LAST_RESULTS = res
    return np.stack([r["y"] for r in res.results], axis=0)
